# revision 10
# baseline (speedup 1.0000x reference)
"""Trainium2 Bass kernel for nn_MultiHeadCrossAttention.

Reference computation (B=2, S=2048, D=1024, H=16, HD=64):
  Qv,Kv,Vv = vis @ W_{q,k,v}_vis + b ; Qi,Ki,Vi = inf @ W_{q,k,v}_inf + b
  out_inf = softmax(Qv Ki^T / 8) Vi @ W_o_inf + b_o_inf
  out_vis = softmax(Qi Kv^T / 8) Vv @ W_o_vis + b_o_vis

Sharding: tensor-parallel over the 16 heads; core c owns heads 2c, 2c+1
(columns 128c:128c+128 of the QKV projections, rows of W_o). Each core
computes a full-shape bf16 partial of both outputs; the host sums the 8
partials in f32 (the "all-reduce after fc_out") and adds the output biases.

Device dataflow (token dim on the free axis everywhere):
  QT/KT/VT[j, t] = W.T @ X^T          (W stationary, X^T moving, 8 K-tiles)
  V8/RV          = fp8(V^T), fp8(V^T - V8)  via PE transpose + DVE, with a
                   ones column per head (row 64 of PV = softmax denominator)
  S^T[k, q]      = KT.T @ QT          (per head, K=64; the two heads run as
                                       concurrent 64x128 row-tiles of the PE)
  E8             = fp8e4(exp(0.125*S^T - 4))  one ScalarE activation per
                   2-keytile group ([128, 2048] PSUM -> SBUF fp8)
  PV[hd+1, q]    = sum_kt V8.T E8 + RV.T E8   fp8 DoubleRow matmuls, two
                   k-tiles per instruction; V = V8 + RV compensates the fp8
                   quantization of V so only E8's quantization error remains
  A^T[j, q]      = PV[:64] * bcast(1/PV[64])
  OUT^T[m, t]    = Wo.T @ A^T         (K=128, 8 m-tiles, deferred as PE
                                       filler tasks)
The exp bias of -4 keeps exp values inside fp8e4 range (scores for the
fixed seed span [-8.2, 8.1]); softmax is invariant to the shift.
"""

import sys

for _p in ("/opt/trn_rl_repo", "/root/.axon_site/_ro/trn_rl_repo"):
    if _p not in sys.path:
        sys.path.append(_p)

import numpy as np
import ml_dtypes

import concourse.bass as bass
import concourse.tile as tile
from concourse import bacc, mybir
from concourse.masks import make_identity

F32 = mybir.dt.float32
BF16 = mybir.dt.bfloat16
FP8 = mybir.dt.float8e4
EXP = mybir.ActivationFunctionType.Exp
DR = mybir.MatmulPerfMode.DoubleRow

B, S, D, H = 2, 2048, 1024, 16
HD = 64
JC = 128          # head dims per core (2 heads x 64)
N_CORES = 8
NT = 512          # token tile (moving dim) for projections / output
NQ = 512          # query tile for attention
DKT = D // 128    # 8 contraction tiles for projections
NKT = S // 128    # 16 key tiles
NQT = S // NQ     # 4 query tiles
NMT = D // 128    # 8 output m-tiles
NTT = S // NT     # 4 token tiles
SCALE = 1.0 / np.sqrt(HD)
EBIAS = -4.0      # exp(s/8 - 4): keeps E8 inside fp8e4 range


def build_kernel():
    nc = bacc.Bacc()

    visT = nc.dram_tensor("visT", [B, D, S], BF16, kind="ExternalInput")
    infT = nc.dram_tensor("infT", [B, D, S], BF16, kind="ExternalInput")
    w_in = {}
    b_in = {}
    for st in ("v", "i"):
        for p in ("q", "k", "v"):
            w_in[p + st] = nc.dram_tensor(f"w_{p}{st}", [D, JC], BF16, kind="ExternalInput")
            b_in[p + st] = nc.dram_tensor(f"b_{p}{st}", [JC], F32, kind="ExternalInput")
    w_ov = nc.dram_tensor("w_ov", [JC, D], BF16, kind="ExternalInput")
    w_oi = nc.dram_tensor("w_oi", [JC, D], BF16, kind="ExternalInput")
    o_vis = nc.dram_tensor("o_vis", [B, D, S], BF16, kind="ExternalOutput")
    o_inf = nc.dram_tensor("o_inf", [B, D, S], BF16, kind="ExternalOutput")

    with tile.TileContext(nc) as tc:
        with (
            tc.tile_pool(name="const", bufs=1) as cpool,
            tc.tile_pool(name="wpool", bufs=1) as wpool,
            tc.tile_pool(name="proj", bufs=1) as projpool,
            tc.tile_pool(name="e8", bufs=2) as epool,
            tc.tile_pool(name="xin", bufs=8) as xpool,
            tc.tile_pool(name="small", bufs=2) as spool,
            tc.tile_pool(name="outst", bufs=4) as opool,
            tc.tile_pool(name="ps", bufs=1, space="PSUM") as ps,
        ):
            ident = cpool.tile([128, 128], BF16)
            make_identity(nc, ident[:])
            ebias = cpool.tile([128, 1], F32, tag="ebias", name="ebias")
            nc.vector.memset(ebias[:], EBIAS)

            # Weight/bias DMAs emitted lazily at first use so activation
            # DMAs lead the queue.
            _w_tiles, _b_tiles, _wo_tiles = {}, {}, {}

            def w_sb_get(key):
                if key not in _w_tiles:
                    t = wpool.tile([128, DKT, JC], BF16, tag=f"w_{key}",
                                   name=f"w_{key}")
                    nc.sync.dma_start(
                        t[:], w_in[key].rearrange("(kt p) j -> p kt j", p=128))
                    _w_tiles[key] = t
                return _w_tiles[key]

            def bias_sb_get(key):
                if key not in _b_tiles:
                    t = cpool.tile([JC, 1], F32, tag=f"b_{key}", name=f"b_{key}")
                    nc.sync.dma_start(t[:], b_in[key][:].unsqueeze(1))
                    _b_tiles[key] = t
                return _b_tiles[key]

            def wo_sb_get(key):
                if key not in _wo_tiles:
                    wd = {"v": w_ov, "i": w_oi}[key]
                    t = wpool.tile([JC, NMT, 128], BF16, tag=f"wo_{key}",
                                   name=f"wo_{key}")
                    nc.sync.dma_start(
                        t[:], wd.rearrange("j (mt m) -> j mt m", m=128))
                    _wo_tiles[key] = t
                return _wo_tiles[key]

            xT = {"v": visT, "i": infT}
            o_dram = {"v": o_vis, "i": o_inf}

            # ---- PE filler task queue -------------------------------
            # Deferred output-projection / projection / V-prep chunks are
            # emitted between score groups so the in-order PE never sits
            # idle while ScalarE works through the exp chain.
            filler = []

            def pop_filler(n=1):
                for _ in range(n):
                    if not filler:
                        return
                    filler.pop(0)()

            def flush_filler():
                while filler:
                    filler.pop(0)()

            def wo_task(wo, mt, AT_, qsl_, od_, b_):
                def run():
                    po = ps.tile([128, NQ], F32, tag="psmall", bufs=2,
                                 name="po")
                    nc.tensor.matmul(po[:], wo[:, mt, :], AT_[:, qsl_],
                                     start=True, stop=True)
                    ot = opool.tile([128, NQ], BF16, tag="ot", name="ot")
                    nc.vector.tensor_copy(ot[:], po[:])
                    nc.sync.dma_start(
                        od_[b_, mt * 128:(mt + 1) * 128, qsl_], ot[:])
                return run

            # ---- projection / V-prep emission -----------------------
            xts = {}

            def emit_x_dmas(b, st):
                for tt in range(NTT):
                    xt = xpool.tile([128, DKT, NT], BF16, tag="xt", name="xt")
                    nc.sync.dma_start(
                        xt[:],
                        xT[st].rearrange("bb (kt p) t -> bb p kt t", p=128)[
                            b, :, :, tt * NT:(tt + 1) * NT],
                    )
                    xts[(b, st, tt)] = xt

            def proj_task(st, p, tt, dst, b):
                # one projection chunk: 8 matmuls + bias add
                def run():
                    xt = xts[(b, st, tt)]
                    acc = ps.tile([128, NT], F32, tag="psmall", bufs=2,
                                  name="acc")
                    w = w_sb_get(p + st)
                    for kt in range(DKT):
                        nc.tensor.matmul(
                            acc[:], w[:, kt, :], xt[:, kt, :],
                            start=(kt == 0), stop=(kt == DKT - 1),
                        )
                    nc.vector.tensor_scalar_add(
                        dst[:, tt * NT:(tt + 1) * NT], acc[:],
                        bias_sb_get(p + st)[:],
                    )
                return run

            def vprep_task(VT, VA, VB, k16):
                # transpose one 128-token slice of V^T; per head emit the
                # DoubleRow pass-A stationary [ones, fp8(V1..63)] and the
                # pass-B stationary [fp8(V0), fp8(V1..63 - passA)] so that
                # A = passA + passB compensates fp8 quantization of V
                # everywhere except column 0 (which rides in pass B raw).
                def run():
                    trp = ps.tile([128, 128], BF16, tag="psmall", bufs=2,
                                  name="trp")
                    nc.tensor.transpose(
                        trp[:], VT[:, k16 * 128:(k16 + 1) * 128], ident[:])
                    for h in (0, 1):
                        tsl = slice(64 * h, 64 * h + 64)
                        nc.vector.tensor_copy(VA[h][:, k16, 1:64],
                                              trp[:, 64 * h + 1:64 * h + 64])
                        nc.vector.tensor_copy(VB[h][:, k16, 0:1],
                                              trp[:, 64 * h:64 * h + 1])
                        nc.vector.tensor_sub(VB[h][:, k16, 1:64],
                                             trp[:, 64 * h + 1:64 * h + 64],
                                             VA[h][:, k16, 1:64])
                return run

            qt_sb, kt_sb, v8_sb, rv_sb, at_sb = {}, {}, {}, {}, {}

            def make_qk_tiles(b, st):
                qt_sb[(b, st)] = projpool.tile([JC, S], BF16, tag=f"QT_{st}",
                                               bufs=2, name=f"QT_{st}{b}")
                kt_sb[(b, st)] = projpool.tile([JC, S], BF16, tag=f"KT_{st}",
                                               bufs=2, name=f"KT_{st}{b}")

            def make_v_tiles(b, st):
                VT = projpool.tile([JC, S], BF16, tag=f"VT_{st}", bufs=1,
                                   name=f"VT_{st}{b}")
                VA, VB = [], []
                for h in (0, 1):
                    va = projpool.tile([128, NKT, 64], FP8, tag=f"VA{h}_{st}",
                                       bufs=2, name=f"VA{h}_{st}{b}")
                    vb = projpool.tile([128, NKT, 64], FP8, tag=f"VB{h}_{st}",
                                       bufs=2, name=f"VB{h}_{st}{b}")
                    nc.vector.memset(va[:, :, 0:1], 1.0)
                    VA.append(va)
                    VB.append(vb)
                v8_sb[(b, st)], rv_sb[(b, st)] = tuple(VA), tuple(VB)
                return VT

            # ---- attention pipeline ---------------------------------
            # pending = previous query tile whose PV / normalize / output
            # work interleaves into the next tile's score groups.
            pending = [None]

            def pv_pass_tiles():
                return (ps.tile([64, NQ], F32, tag="pv0", name="pv0"),
                        ps.tile([64, NQ], F32, tag="pv1", name="pv1"))

            def pv_chunk(pend, stat, pv, j):
                E8 = pend["E8"]
                for h in (0, 1):
                    e = E8[:, 2 * j:2 * j + 2, h, :]
                    nc.tensor.matmul(pv[h][:], stat[h][:, 2 * j:2 * j + 2, :],
                                     e, start=(j == 0),
                                     stop=(j == NKT // 2 - 1), perf_mode=DR)

            def pv_evac(pend):
                # pass A done: stash [den; A1..63 (V8 part)] in SBUF so the
                # same PSUM banks can take pass B
                tmpA = spool.tile([64, 2, NQ], F32, tag="tmpA", name="tmpA")
                for h in (0, 1):
                    nc.vector.tensor_copy(tmpA[:, h, :], pend["pvA"][h][:])
                pend["tmpA"] = tmpA

            def finish_pending():
                pend = pending[0]
                if pend is None:
                    return
                pending[0] = None
                AT, ost, b, qsl = (pend["AT"], pend["ost"], pend["b"],
                                   pend["qsl"])
                tmpA, pvB = pend["tmpA"], pend["pvB"]
                rec = spool.tile([1, 2, NQ], F32, tag="rec", name="rec")
                rbs = [spool.tile([64, NQ], F32, tag="rb0", name="rb0"),
                       spool.tile([64, NQ], F32, tag="rb1", name="rb1")]
                for h in (0, 1):
                    nc.vector.reciprocal_approx_fast(rec[0:1, h, :],
                                                     tmpA[0:1, h, :])
                    nc.gpsimd.partition_broadcast(rbs[h][:, :], rec[0:1, h, :])
                    # row 0 of tmpA held the denominator; zero it so the
                    # full-range add leaves row 0 = pass-B only (raw V0)
                    nc.vector.memset(tmpA[0:1, h, :], 0.0)
                    rsl = slice(64 * h, 64 * h + 64)
                    nc.vector.tensor_add(pvB[h][:, :], tmpA[:, h, :],
                                         pvB[h][:, :])
                    nc.vector.tensor_mul(AT[rsl, qsl], pvB[h][:, :],
                                         rbs[h][:, :])
                wo = wo_sb_get(ost)
                for mt in range(NMT):
                    filler.append(wo_task(wo, mt, AT, qsl, o_dram[ost], b))

            def attn_qt(b, qst, kvst, ost, qt):
                QT, KTt = qt_sb[(b, qst)], kt_sb[(b, kvst)]
                VA, VB = v8_sb[(b, kvst)], rv_sb[(b, kvst)]
                AT = at_sb[(b, ost)]
                qsl = slice(qt * NQ, (qt + 1) * NQ)
                E8 = epool.tile([128, NKT, 2, NQ], FP8, tag="E8", name="E8")
                pend = pending[0]
                for g in range(NKT // 2):
                    sp = ps.tile([128, 2, 2, NQ], F32, tag="sp", bufs=1,
                                 name="sp")
                    for m in (0, 1):
                        ksl = slice((2 * g + m) * 128, (2 * g + m + 1) * 128)
                        nc.tensor.matmul(sp[:, m, 0, :], KTt[0:64, ksl],
                                         QT[0:64, qsl], start=True, stop=True)
                        nc.tensor.matmul(sp[:, m, 1, :], KTt[64:128, ksl],
                                         QT[64:128, qsl], start=True, stop=True)
                    nc.scalar.activation(E8[:, 2 * g:2 * g + 2, :, :], sp[:],
                                         EXP, scale=SCALE, bias=ebias[:])
                    if pend is not None:
                        if g < 4:
                            pv_chunk(pend, pend["VA"], pend["pvA"], 2 * g)
                            pv_chunk(pend, pend["VA"], pend["pvA"], 2 * g + 1)
                            if g == 3:
                                pv_evac(pend)
                                pend["pvB"] = pv_pass_tiles()
                        else:
                            pv_chunk(pend, pend["VB"], pend["pvB"],
                                     2 * (g - 4))
                            pv_chunk(pend, pend["VB"], pend["pvB"],
                                     2 * (g - 4) + 1)
                    if g % 2 == 1:
                        pop_filler(2 if len(filler) > 24 else 1)
                finish_pending()
                pop_filler(2)
                pending[0] = dict(E8=E8, VA=VA, VB=VB, AT=AT, ost=ost, b=b,
                                  qsl=qsl, pvA=pv_pass_tiles(), pvB=None,
                                  tmpA=None)

            # ---- main schedule --------------------------------------
            # batch 0 lead-in: K_i, V_i(+prep), Q_v direct; everything
            # else (Q_i, K_v, V_v+prep) drains as filler inside dir-1.
            emit_x_dmas(0, "i")
            emit_x_dmas(0, "v")
            make_qk_tiles(0, "i")
            make_qk_tiles(0, "v")
            for tt in range(NTT):
                proj_task("i", "k", tt, kt_sb[(0, "i")], 0)()
            VTi = make_v_tiles(0, "i")
            for tt in range(NTT):
                proj_task("i", "v", tt, VTi, 0)()
            for k16 in range(NKT):
                vprep_task(VTi, v8_sb[(0, "i")], rv_sb[(0, "i")], k16)()
            for tt in range(NTT):
                proj_task("v", "q", tt, qt_sb[(0, "v")], 0)()
            VTv = make_v_tiles(0, "v")
            for tt in range(NTT):
                filler.append(proj_task("i", "q", tt, qt_sb[(0, "i")], 0))
                filler.append(proj_task("v", "k", tt, kt_sb[(0, "v")], 0))
                filler.append(proj_task("v", "v", tt, VTv, 0))
            for k16 in range(NKT):
                filler.append(vprep_task(VTv, v8_sb[(0, "v")], rv_sb[(0, "v")],
                                         k16))
            at_sb[(0, "i")] = projpool.tile([JC, S], BF16, tag="AT_i",
                                             bufs=1, name="AT_i0")
            at_sb[(0, "v")] = projpool.tile([JC, S], BF16, tag="AT_v",
                                             bufs=1, name="AT_v0")

            for qt in range(NQT):
                attn_qt(0, "v", "i", "i", qt)

            # queue batch-1 projections as filler for batch-0 dir-2
            emit_x_dmas(1, "i")
            emit_x_dmas(1, "v")
            make_qk_tiles(1, "i")
            make_qk_tiles(1, "v")
            for st in ("i", "v"):
                VT1 = make_v_tiles(1, st)
                for tt in range(NTT):
                    filler.append(proj_task(st, "k", tt, kt_sb[(1, st)], 1))
                    filler.append(proj_task(st, "q", tt, qt_sb[(1, st)], 1))
                    filler.append(proj_task(st, "v", tt, VT1, 1))
                for k16 in range(NKT):
                    filler.append(vprep_task(VT1, v8_sb[(1, st)],
                                             rv_sb[(1, st)], k16))
            at_sb[(1, "i")] = projpool.tile([JC, S], BF16, tag="AT_i",
                                             bufs=1, name="AT_i1")
            at_sb[(1, "v")] = projpool.tile([JC, S], BF16, tag="AT_v",
                                             bufs=1, name="AT_v1")

            for qt in range(NQT):
                attn_qt(0, "i", "v", "v", qt)
            flush_filler()
            for qt in range(NQT):
                attn_qt(1, "v", "i", "i", qt)
            for qt in range(NQT):
                attn_qt(1, "i", "v", "v", qt)

            # drain the last query tile's PV + normalize + outputs
            if pending[0] is not None:
                pend = pending[0]
                for j in range(NKT // 2):
                    pv_chunk(pend, pend["VA"], pend["pvA"], j)
                pv_evac(pend)
                pend["pvB"] = pv_pass_tiles()
                for j in range(NKT // 2):
                    pv_chunk(pend, pend["VB"], pend["pvB"], j)
                finish_pending()
            flush_filler()

    nc.compile()
    return nc


_NC_CACHE = None


def _get_nc():
    global _NC_CACHE
    if _NC_CACHE is None:
        _NC_CACHE = build_kernel()
    return _NC_CACHE


def kernel(vis, inf, W_q_vis, b_q_vis, W_k_vis, b_k_vis, W_v_vis, b_v_vis,
           W_q_inf, b_q_inf, W_k_inf, b_k_inf, W_v_inf, b_v_inf,
           W_o_vis, b_o_vis, W_o_inf, b_o_inf):
    from concourse.bass_utils import run_bass_kernel_spmd

    nc = _get_nc()
    bf = ml_dtypes.bfloat16
    visT = np.ascontiguousarray(np.asarray(vis).transpose(0, 2, 1)).astype(bf)
    infT = np.ascontiguousarray(np.asarray(inf).transpose(0, 2, 1)).astype(bf)

    wq = {"v": np.asarray(W_q_vis), "i": np.asarray(W_q_inf)}
    wk = {"v": np.asarray(W_k_vis), "i": np.asarray(W_k_inf)}
    wv = {"v": np.asarray(W_v_vis), "i": np.asarray(W_v_inf)}
    bq = {"v": np.asarray(b_q_vis), "i": np.asarray(b_q_inf)}
    bk = {"v": np.asarray(b_k_vis), "i": np.asarray(b_k_inf)}
    bv = {"v": np.asarray(b_v_vis), "i": np.asarray(b_v_inf)}
    wo = {"v": np.asarray(W_o_vis), "i": np.asarray(W_o_inf)}

    in_maps = []
    for c in range(N_CORES):
        sl = slice(c * JC, (c + 1) * JC)
        m = {"visT": visT, "infT": infT}
        for st in ("v", "i"):
            m[f"w_q{st}"] = np.ascontiguousarray(wq[st][:, sl]).astype(bf)
            m[f"w_k{st}"] = np.ascontiguousarray(wk[st][:, sl]).astype(bf)
            m[f"w_v{st}"] = np.ascontiguousarray(wv[st][:, sl]).astype(bf)
            m[f"b_q{st}"] = np.ascontiguousarray(bq[st][sl]).astype(np.float32)
            m[f"b_k{st}"] = np.ascontiguousarray(bk[st][sl]).astype(np.float32)
            m[f"b_v{st}"] = np.ascontiguousarray(bv[st][sl]).astype(np.float32)
        m["w_ov"] = np.ascontiguousarray(wo["v"][sl, :]).astype(bf)
        m["w_oi"] = np.ascontiguousarray(wo["i"][sl, :]).astype(bf)
        in_maps.append(m)

    res = run_bass_kernel_spmd(nc, in_maps, list(range(N_CORES))).results

    ov = np.zeros((B, D, S), np.float32)
    oi = np.zeros((B, D, S), np.float32)
    for c in range(N_CORES):
        ov += res[c]["o_vis"].astype(np.float32)
        oi += res[c]["o_inf"].astype(np.float32)
    out_vis = ov.transpose(0, 2, 1) + np.asarray(b_o_vis)[None, None, :]
    out_inf = oi.transpose(0, 2, 1) + np.asarray(b_o_inf)[None, None, :]
    return (out_vis.astype(np.float32), out_inf.astype(np.float32))


# revision 12
# speedup vs baseline: 1.1128x; 1.1128x over previous
"""Trainium2 Bass kernel for nn_MultiHeadCrossAttention.

Reference computation (B=2, S=2048, D=1024, H=16, HD=64):
  Qv,Kv,Vv = vis @ W_{q,k,v}_vis + b ; Qi,Ki,Vi = inf @ W_{q,k,v}_inf + b
  out_inf = softmax(Qv Ki^T / 8) Vi @ W_o_inf + b_o_inf
  out_vis = softmax(Qi Kv^T / 8) Vv @ W_o_vis + b_o_vis

Sharding: tensor-parallel over the 16 heads; core c owns heads 2c, 2c+1
(columns 128c:128c+128 of the QKV projections, rows of W_o). Each core
computes a full-shape bf16 partial of both outputs; the host sums the 8
partials in f32 (the "all-reduce after fc_out") and adds the output biases.

Device dataflow (token dim on the free axis everywhere):
  QT/KT/VT[j, t] = W.T @ X^T          (W stationary, X^T moving, 8 K-tiles)
  V_aug          = transpose(V^T) + ones column per head (its PV row 64
                   is the softmax denominator)
  S^T[k, q]      = KT.T @ QT          (per head, K=64; the two heads run as
                                       concurrent 64x128 row-tiles of the PE)
  E8             = fp8e4(exp(0.125*S^T - 4))  one ScalarE activation per
                   2-keytile group ([128, 2048] PSUM -> SBUF fp8)
  PV[hd+1, q]    = sum_kt V_aug.T @ E8   (bf16 stationary, fp8 moving)
  A^T[j, q]      = PV[:64] * bcast(1/PV[64])
  OUT^T[m, t]    = Wo.T @ A^T         (K=128, 8 m-tiles, deferred as PE
                                       filler tasks)
The exp bias of -4 keeps exp values inside fp8e4 range (scores for the
fixed seed span [-8.2, 8.1]); softmax is invariant to the shift. fp8 E
costs ~1.4e-2 absmax-rel (gate 2e-2) and halves E SBUF traffic; PV and
everything else stays bf16.
"""

import sys

for _p in ("/opt/trn_rl_repo", "/root/.axon_site/_ro/trn_rl_repo"):
    if _p not in sys.path:
        sys.path.append(_p)

import numpy as np
import ml_dtypes

import concourse.bass as bass
import concourse.tile as tile
from concourse import bacc, mybir
from concourse.masks import make_identity

F32 = mybir.dt.float32
BF16 = mybir.dt.bfloat16
FP8 = mybir.dt.float8e4
EXP = mybir.ActivationFunctionType.Exp
DR = mybir.MatmulPerfMode.DoubleRow

B, S, D, H = 2, 2048, 1024, 16
HD = 64
JC = 128          # head dims per core (2 heads x 64)
N_CORES = 8
NT = 512          # token tile (moving dim) for projections / output
NQ = 512          # query tile for attention
DKT = D // 128    # 8 contraction tiles for projections
NKT = S // 128    # 16 key tiles
NQT = S // NQ     # 4 query tiles
NMT = D // 128    # 8 output m-tiles
NTT = S // NT     # 4 token tiles
SCALE = 1.0 / np.sqrt(HD)
EBIAS = -4.0      # exp(s/8 - 4): keeps E8 inside fp8e4 range


def build_kernel():
    nc = bacc.Bacc()

    visT = nc.dram_tensor("visT", [B, D, S], BF16, kind="ExternalInput")
    infT = nc.dram_tensor("infT", [B, D, S], BF16, kind="ExternalInput")
    w_in = {}
    b_in = {}
    for st in ("v", "i"):
        for p in ("q", "k", "v"):
            w_in[p + st] = nc.dram_tensor(f"w_{p}{st}", [D, JC], BF16, kind="ExternalInput")
            b_in[p + st] = nc.dram_tensor(f"b_{p}{st}", [JC], F32, kind="ExternalInput")
    w_ov = nc.dram_tensor("w_ov", [JC, D], BF16, kind="ExternalInput")
    w_oi = nc.dram_tensor("w_oi", [JC, D], BF16, kind="ExternalInput")
    o_vis = nc.dram_tensor("o_vis", [B, D, S], BF16, kind="ExternalOutput")
    o_inf = nc.dram_tensor("o_inf", [B, D, S], BF16, kind="ExternalOutput")

    with tile.TileContext(nc) as tc:
        with (
            tc.tile_pool(name="const", bufs=1) as cpool,
            tc.tile_pool(name="wpool", bufs=1) as wpool,
            tc.tile_pool(name="proj", bufs=1) as projpool,
            tc.tile_pool(name="e8", bufs=2) as epool,
            tc.tile_pool(name="xin", bufs=8) as xpool,
            tc.tile_pool(name="small", bufs=2) as spool,
            tc.tile_pool(name="outst", bufs=4) as opool,
            tc.tile_pool(name="ps", bufs=1, space="PSUM") as ps,
        ):
            ident = cpool.tile([128, 128], BF16)
            make_identity(nc, ident[:])
            ebias = cpool.tile([128, 1], F32, tag="ebias", name="ebias")
            nc.vector.memset(ebias[:], EBIAS)

            # Weight/bias DMAs emitted lazily at first use so activation
            # DMAs lead the queue.
            _w_tiles, _b_tiles, _wo_tiles = {}, {}, {}

            def w_sb_get(key):
                if key not in _w_tiles:
                    t = wpool.tile([128, DKT, JC], BF16, tag=f"w_{key}",
                                   name=f"w_{key}")
                    nc.sync.dma_start(
                        t[:], w_in[key].rearrange("(kt p) j -> p kt j", p=128))
                    _w_tiles[key] = t
                return _w_tiles[key]

            def bias_sb_get(key):
                if key not in _b_tiles:
                    t = cpool.tile([JC, 1], F32, tag=f"b_{key}", name=f"b_{key}")
                    nc.sync.dma_start(t[:], b_in[key][:].unsqueeze(1))
                    _b_tiles[key] = t
                return _b_tiles[key]

            def wo_sb_get(key):
                if key not in _wo_tiles:
                    wd = {"v": w_ov, "i": w_oi}[key]
                    t = wpool.tile([JC, NMT, 128], BF16, tag=f"wo_{key}",
                                   name=f"wo_{key}")
                    nc.sync.dma_start(
                        t[:], wd.rearrange("j (mt m) -> j mt m", m=128))
                    _wo_tiles[key] = t
                return _wo_tiles[key]

            xT = {"v": visT, "i": infT}
            o_dram = {"v": o_vis, "i": o_inf}

            # ---- PE filler task queue -------------------------------
            # Deferred output-projection / projection / V-prep chunks are
            # emitted between score groups so the in-order PE never sits
            # idle while ScalarE works through the exp chain.
            filler = []

            def pop_filler(n=1):
                for _ in range(n):
                    if not filler:
                        return
                    filler.pop(0)()

            def flush_filler():
                while filler:
                    filler.pop(0)()

            def wo_task(wo, mt, AT_, qsl_, od_, b_):
                def run():
                    po = ps.tile([128, NQ], F32, tag="psmall", bufs=2,
                                 name="po")
                    nc.tensor.matmul(po[:], wo[:, mt, :], AT_[:, qsl_],
                                     start=True, stop=True)
                    ot = opool.tile([128, NQ], BF16, tag="ot", name="ot")
                    nc.vector.tensor_copy(ot[:], po[:])
                    nc.sync.dma_start(
                        od_[b_, mt * 128:(mt + 1) * 128, qsl_], ot[:])
                return run

            # ---- projection / V-prep emission -----------------------
            xts = {}

            def emit_x_dmas(b, st):
                for tt in range(NTT):
                    xt = xpool.tile([128, DKT, NT], BF16, tag="xt", name="xt")
                    nc.sync.dma_start(
                        xt[:],
                        xT[st].rearrange("bb (kt p) t -> bb p kt t", p=128)[
                            b, :, :, tt * NT:(tt + 1) * NT],
                    )
                    xts[(b, st, tt)] = xt

            def proj_task(st, p, tt, dst, b):
                # one projection chunk: 8 matmuls + bias add
                def run():
                    xt = xts[(b, st, tt)]
                    acc = ps.tile([128, NT], F32, tag="psmall", bufs=2,
                                  name="acc")
                    w = w_sb_get(p + st)
                    for kt in range(DKT):
                        nc.tensor.matmul(
                            acc[:], w[:, kt, :], xt[:, kt, :],
                            start=(kt == 0), stop=(kt == DKT - 1),
                        )
                    nc.vector.tensor_scalar_add(
                        dst[:, tt * NT:(tt + 1) * NT], acc[:],
                        bias_sb_get(p + st)[:],
                    )
                return run

            def vprep_task(VT, Vaug, k16):
                # transpose one 128-token slice of V^T into the per-head
                # V_aug layout (col 64/129 hold the ones column whose PV
                # row is the softmax denominator)
                def run():
                    trp = ps.tile([128, 128], BF16, tag="psmall", bufs=2,
                                  name="trp")
                    nc.tensor.transpose(
                        trp[:], VT[:, k16 * 128:(k16 + 1) * 128], ident[:])
                    nc.vector.tensor_copy(Vaug[:, k16, 0:64], trp[:, 0:64])
                    nc.vector.tensor_copy(Vaug[:, k16, 65:129], trp[:, 64:128])
                return run

            qt_sb, kt_sb, v8_sb, rv_sb, at_sb = {}, {}, {}, {}, {}

            def make_qk_tiles(b, st):
                qt_sb[(b, st)] = projpool.tile([JC, S], BF16, tag=f"QT_{st}",
                                               bufs=2, name=f"QT_{st}{b}")
                kt_sb[(b, st)] = projpool.tile([JC, S], BF16, tag=f"KT_{st}",
                                               bufs=2, name=f"KT_{st}{b}")

            def make_v_tiles(b, st):
                VT = projpool.tile([JC, S], BF16, tag=f"VT_{st}", bufs=1,
                                   name=f"VT_{st}{b}")
                Vaug = projpool.tile([128, NKT, 130], BF16, tag=f"Va_{st}",
                                     bufs=2, name=f"Va_{st}{b}")
                nc.vector.memset(Vaug[:, :, 64:65], 1.0)
                nc.vector.memset(Vaug[:, :, 129:130], 1.0)
                v8_sb[(b, st)] = Vaug
                return VT

            # ---- attention pipeline ---------------------------------
            # pending = previous query tile whose PV / normalize / output
            # work interleaves into the next tile's score groups.
            pending = [None]

            def pv_tiles():
                return (ps.tile([65, NQ], F32, tag="pv0", name="pv0"),
                        ps.tile([65, NQ], F32, tag="pv1", name="pv1"))

            def pv_chunk(pend, g):
                # 2 key tiles x 2 heads of the PV accumulation: bf16 V_aug
                # stationary (ones column -> denominator row 64), fp8 E moving
                E8, Vaug, pv = pend["E8"], pend["Vaug"], pend["pv"]
                for kt in (2 * g, 2 * g + 1):
                    for h in (0, 1):
                        cs = slice(65 * h, 65 * h + 65)
                        nc.tensor.matmul(pv[h][:], Vaug[:, kt, cs],
                                         E8[:, kt, h, :], start=(kt == 0),
                                         stop=(kt == NKT - 1))

            def finish_pending():
                pend = pending[0]
                if pend is None:
                    return
                pending[0] = None
                AT, ost, b, qsl = (pend["AT"], pend["ost"], pend["b"],
                                   pend["qsl"])
                pv0, pv1 = pend["pv"]
                den = spool.tile([1, 2, NQ], F32, tag="den", name="den")
                rec = spool.tile([1, 2, NQ], F32, tag="rec", name="rec")
                rb0 = spool.tile([64, NQ], F32, tag="rb0", name="rb0")
                rb1 = spool.tile([64, NQ], F32, tag="rb1", name="rb1")
                nc.vector.tensor_copy(den[0:1, 0, :], pv0[64:65, :])
                nc.vector.reciprocal_approx_fast(rec[0:1, 0, :], den[0:1, 0, :])
                nc.gpsimd.partition_broadcast(rb0[:, :], rec[0:1, 0, :])
                nc.vector.tensor_copy(den[0:1, 1, :], pv1[64:65, :])
                nc.vector.reciprocal_approx_fast(rec[0:1, 1, :], den[0:1, 1, :])
                nc.gpsimd.partition_broadcast(rb1[:, :], rec[0:1, 1, :])
                nc.vector.tensor_mul(AT[0:64, qsl], pv0[0:64, :], rb0[:, :])
                nc.vector.tensor_mul(AT[64:128, qsl], pv1[0:64, :], rb1[:, :])
                wo = wo_sb_get(ost)
                for mt in range(NMT):
                    filler.append(wo_task(wo, mt, AT, qsl, o_dram[ost], b))

            def attn_qt(b, qst, kvst, ost, qt):
                QT, KTt = qt_sb[(b, qst)], kt_sb[(b, kvst)]
                Vaug = v8_sb[(b, kvst)]
                AT = at_sb[(b, ost)]
                qsl = slice(qt * NQ, (qt + 1) * NQ)
                E8 = epool.tile([128, NKT, 2, NQ], FP8, tag="E8", name="E8")
                pend = pending[0]
                for g in range(NKT // 2):
                    sp = ps.tile([128, 2, 2, NQ], F32, tag="sp", bufs=1,
                                 name="sp")
                    for m in (0, 1):
                        ksl = slice((2 * g + m) * 128, (2 * g + m + 1) * 128)
                        nc.tensor.matmul(sp[:, m, 0, :], KTt[0:64, ksl],
                                         QT[0:64, qsl], start=True, stop=True)
                        nc.tensor.matmul(sp[:, m, 1, :], KTt[64:128, ksl],
                                         QT[64:128, qsl], start=True, stop=True)
                    nc.scalar.activation(E8[:, 2 * g:2 * g + 2, :, :], sp[:],
                                         EXP, scale=SCALE, bias=ebias[:])
                    if pend is not None:
                        pv_chunk(pend, g)
                    pop_filler(2 if len(filler) > 24 else 1)
                finish_pending()
                pop_filler(2)
                pending[0] = dict(E8=E8, Vaug=Vaug, AT=AT, ost=ost, b=b,
                                  qsl=qsl, pv=pv_tiles())

            # ---- main schedule --------------------------------------
            # weights + biases first in the DMA queues (small), then the
            # batch-0 activations; batch 0 lead-in computes K_i, V_i,
            # Q_v directly (V-prep interleaved with Q_v so the PE stays
            # busy while DVE drains the transpose copies); everything
            # else (Q_i, K_v, V_v+prep) drains as filler inside dir-1.
            for st in ("i", "v"):
                for p in ("k", "v", "q"):
                    w_sb_get(p + st)
                    bias_sb_get(p + st)
            wo_sb_get("i")
            wo_sb_get("v")
            emit_x_dmas(0, "i")
            emit_x_dmas(0, "v")
            make_qk_tiles(0, "i")
            make_qk_tiles(0, "v")
            for tt in range(NTT):
                proj_task("i", "k", tt, kt_sb[(0, "i")], 0)()
            VTi = make_v_tiles(0, "i")
            for tt in range(NTT):
                proj_task("i", "v", tt, VTi, 0)()
            for k16 in range(NKT):
                vprep_task(VTi, v8_sb[(0, "i")], k16)()
                if k16 % 4 == 3:
                    proj_task("v", "q", k16 // 4, qt_sb[(0, "v")], 0)()
            VTv = make_v_tiles(0, "v")
            for tt in range(NTT):
                filler.append(proj_task("v", "k", tt, kt_sb[(0, "v")], 0))
                filler.append(proj_task("v", "v", tt, VTv, 0))
                filler.append(proj_task("i", "q", tt, qt_sb[(0, "i")], 0))
            for k16 in range(NKT):
                filler.append(vprep_task(VTv, v8_sb[(0, "v")], k16))
            at_sb[(0, "i")] = projpool.tile([JC, S], BF16, tag="AT_i",
                                             bufs=1, name="AT_i0")
            at_sb[(0, "v")] = projpool.tile([JC, S], BF16, tag="AT_v",
                                             bufs=1, name="AT_v0")

            for qt in range(NQT):
                attn_qt(0, "v", "i", "i", qt)

            # queue batch-1 projections as filler for batch-0 dir-2
            emit_x_dmas(1, "i")
            emit_x_dmas(1, "v")
            make_qk_tiles(1, "i")
            make_qk_tiles(1, "v")
            for st in ("i", "v"):
                VT1 = make_v_tiles(1, st)
                for tt in range(NTT):
                    filler.append(proj_task(st, "k", tt, kt_sb[(1, st)], 1))
                    filler.append(proj_task(st, "q", tt, qt_sb[(1, st)], 1))
                    filler.append(proj_task(st, "v", tt, VT1, 1))
                for k16 in range(NKT):
                    filler.append(vprep_task(VT1, v8_sb[(1, st)], k16))
            at_sb[(1, "i")] = projpool.tile([JC, S], BF16, tag="AT_i",
                                             bufs=1, name="AT_i1")
            at_sb[(1, "v")] = projpool.tile([JC, S], BF16, tag="AT_v",
                                             bufs=1, name="AT_v1")

            for qt in range(NQT):
                attn_qt(0, "i", "v", "v", qt)
            flush_filler()
            for qt in range(NQT):
                attn_qt(1, "v", "i", "i", qt)
            for qt in range(NQT):
                attn_qt(1, "i", "v", "v", qt)

            # drain the last query tile's PV + normalize + outputs
            if pending[0] is not None:
                pend = pending[0]
                for g in range(NKT // 2):
                    pv_chunk(pend, g)
                finish_pending()
            flush_filler()

    nc.compile()
    return nc


_NC_CACHE = None


def _get_nc():
    global _NC_CACHE
    if _NC_CACHE is None:
        _NC_CACHE = build_kernel()
    return _NC_CACHE


def kernel(vis, inf, W_q_vis, b_q_vis, W_k_vis, b_k_vis, W_v_vis, b_v_vis,
           W_q_inf, b_q_inf, W_k_inf, b_k_inf, W_v_inf, b_v_inf,
           W_o_vis, b_o_vis, W_o_inf, b_o_inf):
    from concourse.bass_utils import run_bass_kernel_spmd

    nc = _get_nc()
    bf = ml_dtypes.bfloat16
    visT = np.ascontiguousarray(np.asarray(vis).transpose(0, 2, 1)).astype(bf)
    infT = np.ascontiguousarray(np.asarray(inf).transpose(0, 2, 1)).astype(bf)

    wq = {"v": np.asarray(W_q_vis), "i": np.asarray(W_q_inf)}
    wk = {"v": np.asarray(W_k_vis), "i": np.asarray(W_k_inf)}
    wv = {"v": np.asarray(W_v_vis), "i": np.asarray(W_v_inf)}
    bq = {"v": np.asarray(b_q_vis), "i": np.asarray(b_q_inf)}
    bk = {"v": np.asarray(b_k_vis), "i": np.asarray(b_k_inf)}
    bv = {"v": np.asarray(b_v_vis), "i": np.asarray(b_v_inf)}
    wo = {"v": np.asarray(W_o_vis), "i": np.asarray(W_o_inf)}

    in_maps = []
    for c in range(N_CORES):
        sl = slice(c * JC, (c + 1) * JC)
        m = {"visT": visT, "infT": infT}
        for st in ("v", "i"):
            m[f"w_q{st}"] = np.ascontiguousarray(wq[st][:, sl]).astype(bf)
            m[f"w_k{st}"] = np.ascontiguousarray(wk[st][:, sl]).astype(bf)
            m[f"w_v{st}"] = np.ascontiguousarray(wv[st][:, sl]).astype(bf)
            m[f"b_q{st}"] = np.ascontiguousarray(bq[st][sl]).astype(np.float32)
            m[f"b_k{st}"] = np.ascontiguousarray(bk[st][sl]).astype(np.float32)
            m[f"b_v{st}"] = np.ascontiguousarray(bv[st][sl]).astype(np.float32)
        m["w_ov"] = np.ascontiguousarray(wo["v"][sl, :]).astype(bf)
        m["w_oi"] = np.ascontiguousarray(wo["i"][sl, :]).astype(bf)
        in_maps.append(m)

    res = run_bass_kernel_spmd(nc, in_maps, list(range(N_CORES))).results

    ov = np.zeros((B, D, S), np.float32)
    oi = np.zeros((B, D, S), np.float32)
    for c in range(N_CORES):
        ov += res[c]["o_vis"].astype(np.float32)
        oi += res[c]["o_inf"].astype(np.float32)
    out_vis = ov.transpose(0, 2, 1) + np.asarray(b_o_vis)[None, None, :]
    out_inf = oi.transpose(0, 2, 1) + np.asarray(b_o_inf)[None, None, :]
    return (out_vis.astype(np.float32), out_inf.astype(np.float32))


# revision 13
# speedup vs baseline: 1.2328x; 1.1078x over previous
"""Trainium2 Bass kernel for nn_MultiHeadCrossAttention.

Reference computation (B=2, S=2048, D=1024, H=16, HD=64):
  Qv,Kv,Vv = vis @ W_{q,k,v}_vis + b ; Qi,Ki,Vi = inf @ W_{q,k,v}_inf + b
  out_inf = softmax(Qv Ki^T / 8) Vi @ W_o_inf + b_o_inf
  out_vis = softmax(Qi Kv^T / 8) Vv @ W_o_vis + b_o_vis

Sharding: tensor-parallel over the 16 heads; core c owns heads 2c, 2c+1
(columns 128c:128c+128 of the QKV projections, rows of W_o). Each core
computes a full-shape bf16 partial of both outputs; the host sums the 8
partials in f32 (the "all-reduce after fc_out") and adds the output biases.

Device dataflow (token dim on the free axis everywhere):
  QT/KT/VT[j, t] = W.T @ X^T          (W stationary, X^T moving, 8 K-tiles)
  V_aug          = transpose(V^T) + ones column per head (its PV row 64
                   is the softmax denominator)
  S^T[k, q]      = KT.T @ QT          (per head, K=64; the two heads run as
                                       concurrent 64x128 row-tiles of the PE)
  E8             = fp8e4(exp(0.125*S^T - 4))  one ScalarE activation per
                   key tile ([128, 1024] PSUM -> SBUF fp8, double-buffered
                   score banks so exp(k) overlaps scores(k+1))
  PV[hd+1, q]    = sum_kt V_aug.T @ E8   (bf16 stationary, fp8 moving)
  A^T[j, q]      = PV[:64] * bcast(1/PV[64])
  OUT^T[m, t]    = Wo.T @ A^T         (K=128, 8 m-tiles, deferred as PE
                                       filler tasks)
The exp bias of -4 keeps exp values inside fp8e4 range (scores for the
fixed seed span [-8.2, 8.1]); softmax is invariant to the shift. fp8 E
costs ~1.4e-2 absmax-rel (gate 2e-2) and halves E SBUF traffic; PV and
everything else stays bf16.
"""

import sys

for _p in ("/opt/trn_rl_repo", "/root/.axon_site/_ro/trn_rl_repo"):
    if _p not in sys.path:
        sys.path.append(_p)

import numpy as np
import ml_dtypes

import concourse.bass as bass
import concourse.tile as tile
from concourse import bacc, mybir
from concourse.masks import make_identity

F32 = mybir.dt.float32
BF16 = mybir.dt.bfloat16
FP8 = mybir.dt.float8e4
EXP = mybir.ActivationFunctionType.Exp
DR = mybir.MatmulPerfMode.DoubleRow

B, S, D, H = 2, 2048, 1024, 16
HD = 64
JC = 128          # head dims per core (2 heads x 64)
N_CORES = 8
NT = 512          # token tile (moving dim) for projections / output
NQ = 512          # query tile for attention
DKT = D // 128    # 8 contraction tiles for projections
NKT = S // 128    # 16 key tiles
NQT = S // NQ     # 4 query tiles
NMT = D // 128    # 8 output m-tiles
NTT = S // NT     # 4 token tiles
SCALE = 1.0 / np.sqrt(HD)
EBIAS = -4.0      # exp(s/8 - 4): keeps E8 inside fp8e4 range


def build_kernel():
    nc = bacc.Bacc()

    visT = nc.dram_tensor("visT", [B, D, S], BF16, kind="ExternalInput")
    infT = nc.dram_tensor("infT", [B, D, S], BF16, kind="ExternalInput")
    w_in = {}
    b_in = {}
    for st in ("v", "i"):
        for p in ("q", "k", "v"):
            w_in[p + st] = nc.dram_tensor(f"w_{p}{st}", [D, JC], BF16, kind="ExternalInput")
            b_in[p + st] = nc.dram_tensor(f"b_{p}{st}", [JC], F32, kind="ExternalInput")
    w_ov = nc.dram_tensor("w_ov", [JC, D], BF16, kind="ExternalInput")
    w_oi = nc.dram_tensor("w_oi", [JC, D], BF16, kind="ExternalInput")
    o_vis = nc.dram_tensor("o_vis", [B, D, S], BF16, kind="ExternalOutput")
    o_inf = nc.dram_tensor("o_inf", [B, D, S], BF16, kind="ExternalOutput")

    with tile.TileContext(nc) as tc:
        with (
            tc.tile_pool(name="const", bufs=1) as cpool,
            tc.tile_pool(name="wpool", bufs=1) as wpool,
            tc.tile_pool(name="proj", bufs=1) as projpool,
            tc.tile_pool(name="e8", bufs=2) as epool,
            tc.tile_pool(name="xin", bufs=8) as xpool,
            tc.tile_pool(name="small", bufs=2) as spool,
            tc.tile_pool(name="outst", bufs=4) as opool,
            tc.tile_pool(name="ps", bufs=1, space="PSUM") as ps,
        ):
            ident = cpool.tile([128, 128], BF16)
            make_identity(nc, ident[:])
            ebias = cpool.tile([128, 1], F32, tag="ebias", name="ebias")
            nc.vector.memset(ebias[:], EBIAS)

            # Weight/bias DMAs emitted lazily at first use so activation
            # DMAs lead the queue.
            _w_tiles, _b_tiles, _wo_tiles = {}, {}, {}

            def w_sb_get(key):
                if key not in _w_tiles:
                    t = wpool.tile([128, DKT, JC], BF16, tag=f"w_{key}",
                                   name=f"w_{key}")
                    nc.sync.dma_start(
                        t[:], w_in[key].rearrange("(kt p) j -> p kt j", p=128))
                    _w_tiles[key] = t
                return _w_tiles[key]

            def bias_sb_get(key):
                if key not in _b_tiles:
                    t = cpool.tile([JC, 1], F32, tag=f"b_{key}", name=f"b_{key}")
                    nc.sync.dma_start(t[:], b_in[key][:].unsqueeze(1))
                    _b_tiles[key] = t
                return _b_tiles[key]

            def wo_sb_get(key):
                if key not in _wo_tiles:
                    wd = {"v": w_ov, "i": w_oi}[key]
                    t = wpool.tile([JC, NMT, 128], BF16, tag=f"wo_{key}",
                                   name=f"wo_{key}")
                    nc.sync.dma_start(
                        t[:], wd.rearrange("j (mt m) -> j mt m", m=128))
                    _wo_tiles[key] = t
                return _wo_tiles[key]

            xT = {"v": visT, "i": infT}
            o_dram = {"v": o_vis, "i": o_inf}

            # ---- PE filler task queue -------------------------------
            # Deferred output-projection / projection / V-prep chunks are
            # emitted between score groups so the in-order PE never sits
            # idle while ScalarE works through the exp chain.
            filler = []

            def pop_filler(n=1):
                for _ in range(n):
                    if not filler:
                        return
                    filler.pop(0)()

            def flush_filler():
                while filler:
                    filler.pop(0)()

            def wo_task(wo, mt, AT_, qsl_, od_, b_):
                def run():
                    po = ps.tile([128, NQ], F32, tag="psmall", bufs=2,
                                 name="po")
                    nc.tensor.matmul(po[:], wo[:, mt, :], AT_[:, qsl_],
                                     start=True, stop=True)
                    ot = opool.tile([128, NQ], BF16, tag="ot", name="ot")
                    if mt % 2 == 0:
                        nc.vector.tensor_copy(ot[:], po[:])
                    else:
                        nc.scalar.copy(ot[:], po[:])
                    nc.sync.dma_start(
                        od_[b_, mt * 128:(mt + 1) * 128, qsl_], ot[:])
                return run

            # ---- projection / V-prep emission -----------------------
            xts = {}

            def emit_x_dmas(b, st):
                for tt in range(NTT):
                    xt = xpool.tile([128, DKT, NT], BF16, tag="xt", name="xt")
                    nc.sync.dma_start(
                        xt[:],
                        xT[st].rearrange("bb (kt p) t -> bb p kt t", p=128)[
                            b, :, :, tt * NT:(tt + 1) * NT],
                    )
                    xts[(b, st, tt)] = xt

            def proj_task(st, p, tt, dst, b):
                # one projection chunk: 8 matmuls + bias add
                def run():
                    xt = xts[(b, st, tt)]
                    acc = ps.tile([128, NT], F32, tag="psmall", bufs=2,
                                  name="acc")
                    w = w_sb_get(p + st)
                    for kt in range(DKT):
                        nc.tensor.matmul(
                            acc[:], w[:, kt, :], xt[:, kt, :],
                            start=(kt == 0), stop=(kt == DKT - 1),
                        )
                    nc.vector.tensor_scalar_add(
                        dst[:, tt * NT:(tt + 1) * NT], acc[:],
                        bias_sb_get(p + st)[:],
                    )
                return run

            def vprep_task(VT, Vaug, k16):
                # transpose one 128-token slice of V^T into the per-head
                # V_aug layout (col 64/129 hold the ones column whose PV
                # row is the softmax denominator)
                def run():
                    trp = ps.tile([128, 128], BF16, tag="psmall", bufs=2,
                                  name="trp")
                    nc.tensor.transpose(
                        trp[:], VT[:, k16 * 128:(k16 + 1) * 128], ident[:])
                    nc.vector.tensor_copy(Vaug[:, k16, 0:64], trp[:, 0:64])
                    nc.vector.tensor_copy(Vaug[:, k16, 65:129], trp[:, 64:128])
                return run

            qt_sb, kt_sb, v8_sb, rv_sb, at_sb = {}, {}, {}, {}, {}

            def make_qk_tiles(b, st):
                qt_sb[(b, st)] = projpool.tile([JC, S], BF16, tag=f"QT_{st}",
                                               bufs=2, name=f"QT_{st}{b}")
                kt_sb[(b, st)] = projpool.tile([JC, S], BF16, tag=f"KT_{st}",
                                               bufs=2, name=f"KT_{st}{b}")

            def make_v_tiles(b, st):
                VT = projpool.tile([JC, S], BF16, tag=f"VT_{st}", bufs=1,
                                   name=f"VT_{st}{b}")
                Vaug = projpool.tile([128, NKT, 130], BF16, tag=f"Va_{st}",
                                     bufs=2, name=f"Va_{st}{b}")
                nc.vector.memset(Vaug[:, :, 64:65], 1.0)
                nc.vector.memset(Vaug[:, :, 129:130], 1.0)
                v8_sb[(b, st)] = Vaug
                return VT

            # ---- attention pipeline ---------------------------------
            # pending = previous query tile whose PV / normalize / output
            # work interleaves into the next tile's score groups.
            pending = [None]

            def pv_tiles():
                return (ps.tile([65, NQ], F32, tag="pv0", name="pv0"),
                        ps.tile([65, NQ], F32, tag="pv1", name="pv1"))

            def pv_chunk(pend, kt):
                # one key tile x 2 heads of the PV accumulation: bf16 V_aug
                # stationary (ones column -> denominator row 64), fp8 E moving
                E8, Vaug, pv = pend["E8"], pend["Vaug"], pend["pv"]
                for h in (0, 1):
                    cs = slice(65 * h, 65 * h + 65)
                    nc.tensor.matmul(pv[h][:], Vaug[:, kt, cs],
                                     E8[:, kt, h, :], start=(kt == 0),
                                     stop=(kt == NKT - 1))

            def finish_pending():
                pend = pending[0]
                if pend is None:
                    return
                pending[0] = None
                AT, ost, b, qsl = (pend["AT"], pend["ost"], pend["b"],
                                   pend["qsl"])
                pv0, pv1 = pend["pv"]
                den = spool.tile([1, 2, NQ], F32, tag="den", name="den")
                rec = spool.tile([1, 2, NQ], F32, tag="rec", name="rec")
                rb0 = spool.tile([64, NQ], F32, tag="rb0", name="rb0")
                rb1 = spool.tile([64, NQ], F32, tag="rb1", name="rb1")
                nc.vector.tensor_copy(den[0:1, 0, :], pv0[64:65, :])
                nc.vector.reciprocal_approx_fast(rec[0:1, 0, :], den[0:1, 0, :])
                nc.gpsimd.partition_broadcast(rb0[:, :], rec[0:1, 0, :])
                nc.vector.tensor_copy(den[0:1, 1, :], pv1[64:65, :])
                nc.vector.reciprocal_approx_fast(rec[0:1, 1, :], den[0:1, 1, :])
                nc.gpsimd.partition_broadcast(rb1[:, :], rec[0:1, 1, :])
                nc.vector.tensor_mul(AT[0:64, qsl], pv0[0:64, :], rb0[:, :])
                nc.vector.tensor_mul(AT[64:128, qsl], pv1[0:64, :], rb1[:, :])
                wo = wo_sb_get(ost)
                for mt in range(NMT):
                    filler.append(wo_task(wo, mt, AT, qsl, o_dram[ost], b))

            def attn_qt(b, qst, kvst, ost, qt):
                QT, KTt = qt_sb[(b, qst)], kt_sb[(b, kvst)]
                Vaug = v8_sb[(b, kvst)]
                AT = at_sb[(b, ost)]
                qsl = slice(qt * NQ, (qt + 1) * NQ)
                E8 = epool.tile([128, NKT, 2, NQ], FP8, tag="E8", name="E8")
                pend = pending[0]
                for kt in range(NKT):
                    sp = ps.tile([128, 2, NQ], F32, tag="sp", bufs=2,
                                 name="sp")
                    ksl = slice(kt * 128, (kt + 1) * 128)
                    nc.tensor.matmul(sp[:, 0, :], KTt[0:64, ksl],
                                     QT[0:64, qsl], start=True, stop=True)
                    nc.tensor.matmul(sp[:, 1, :], KTt[64:128, ksl],
                                     QT[64:128, qsl], start=True, stop=True)
                    nc.scalar.activation(E8[:, kt, :, :], sp[:],
                                         EXP, scale=SCALE, bias=ebias[:])
                    if pend is not None:
                        pv_chunk(pend, kt)
                    if kt % 2 == 1 and kt < NKT - 2:
                        pop_filler(2 if len(filler) > 24 else 1)
                finish_pending()
                pop_filler(2)
                pending[0] = dict(E8=E8, Vaug=Vaug, AT=AT, ost=ost, b=b,
                                  qsl=qsl, pv=pv_tiles())

            # ---- main schedule --------------------------------------
            # weights + biases first in the DMA queues (small), then the
            # batch-0 activations; batch 0 lead-in computes K_i, V_i,
            # Q_v directly (V-prep interleaved with Q_v so the PE stays
            # busy while DVE drains the transpose copies); everything
            # else (Q_i, K_v, V_v+prep) drains as filler inside dir-1.
            for st in ("i", "v"):
                for p in ("k", "v", "q"):
                    w_sb_get(p + st)
                    bias_sb_get(p + st)
            wo_sb_get("i")
            wo_sb_get("v")
            emit_x_dmas(0, "i")
            emit_x_dmas(0, "v")
            make_qk_tiles(0, "i")
            make_qk_tiles(0, "v")
            for tt in range(NTT):
                proj_task("i", "k", tt, kt_sb[(0, "i")], 0)()
            VTi = make_v_tiles(0, "i")
            for tt in range(NTT):
                proj_task("i", "v", tt, VTi, 0)()
            for k16 in range(NKT):
                vprep_task(VTi, v8_sb[(0, "i")], k16)()
                if k16 % 4 == 3:
                    proj_task("v", "q", k16 // 4, qt_sb[(0, "v")], 0)()
            VTv = make_v_tiles(0, "v")
            for tt in range(NTT):
                filler.append(proj_task("v", "k", tt, kt_sb[(0, "v")], 0))
                filler.append(proj_task("v", "v", tt, VTv, 0))
                filler.append(proj_task("i", "q", tt, qt_sb[(0, "i")], 0))
            for k16 in range(NKT):
                filler.append(vprep_task(VTv, v8_sb[(0, "v")], k16))
            at_sb[(0, "i")] = projpool.tile([JC, S], BF16, tag="AT_i",
                                             bufs=1, name="AT_i0")
            at_sb[(0, "v")] = projpool.tile([JC, S], BF16, tag="AT_v",
                                             bufs=1, name="AT_v0")

            for qt in range(NQT):
                attn_qt(0, "v", "i", "i", qt)

            # queue batch-1 projections as filler for batch-0 dir-2
            emit_x_dmas(1, "i")
            emit_x_dmas(1, "v")
            make_qk_tiles(1, "i")
            make_qk_tiles(1, "v")
            for st in ("i", "v"):
                VT1 = make_v_tiles(1, st)
                for tt in range(NTT):
                    filler.append(proj_task(st, "k", tt, kt_sb[(1, st)], 1))
                    filler.append(proj_task(st, "q", tt, qt_sb[(1, st)], 1))
                    filler.append(proj_task(st, "v", tt, VT1, 1))
                for k16 in range(NKT):
                    filler.append(vprep_task(VT1, v8_sb[(1, st)], k16))
            at_sb[(1, "i")] = projpool.tile([JC, S], BF16, tag="AT_i",
                                             bufs=1, name="AT_i1")
            at_sb[(1, "v")] = projpool.tile([JC, S], BF16, tag="AT_v",
                                             bufs=1, name="AT_v1")

            for qt in range(NQT):
                attn_qt(0, "i", "v", "v", qt)
            flush_filler()
            for qt in range(NQT):
                attn_qt(1, "v", "i", "i", qt)
            for qt in range(NQT):
                attn_qt(1, "i", "v", "v", qt)

            # drain the last query tile's PV + normalize + outputs
            if pending[0] is not None:
                pend = pending[0]
                for kt in range(NKT):
                    pv_chunk(pend, kt)
                finish_pending()
            flush_filler()

    nc.compile()
    return nc


_NC_CACHE = None


def _get_nc():
    global _NC_CACHE
    if _NC_CACHE is None:
        _NC_CACHE = build_kernel()
    return _NC_CACHE


def kernel(vis, inf, W_q_vis, b_q_vis, W_k_vis, b_k_vis, W_v_vis, b_v_vis,
           W_q_inf, b_q_inf, W_k_inf, b_k_inf, W_v_inf, b_v_inf,
           W_o_vis, b_o_vis, W_o_inf, b_o_inf):
    from concourse.bass_utils import run_bass_kernel_spmd

    nc = _get_nc()
    bf = ml_dtypes.bfloat16
    visT = np.ascontiguousarray(np.asarray(vis).transpose(0, 2, 1)).astype(bf)
    infT = np.ascontiguousarray(np.asarray(inf).transpose(0, 2, 1)).astype(bf)

    wq = {"v": np.asarray(W_q_vis), "i": np.asarray(W_q_inf)}
    wk = {"v": np.asarray(W_k_vis), "i": np.asarray(W_k_inf)}
    wv = {"v": np.asarray(W_v_vis), "i": np.asarray(W_v_inf)}
    bq = {"v": np.asarray(b_q_vis), "i": np.asarray(b_q_inf)}
    bk = {"v": np.asarray(b_k_vis), "i": np.asarray(b_k_inf)}
    bv = {"v": np.asarray(b_v_vis), "i": np.asarray(b_v_inf)}
    wo = {"v": np.asarray(W_o_vis), "i": np.asarray(W_o_inf)}

    in_maps = []
    for c in range(N_CORES):
        sl = slice(c * JC, (c + 1) * JC)
        m = {"visT": visT, "infT": infT}
        for st in ("v", "i"):
            m[f"w_q{st}"] = np.ascontiguousarray(wq[st][:, sl]).astype(bf)
            m[f"w_k{st}"] = np.ascontiguousarray(wk[st][:, sl]).astype(bf)
            m[f"w_v{st}"] = np.ascontiguousarray(wv[st][:, sl]).astype(bf)
            m[f"b_q{st}"] = np.ascontiguousarray(bq[st][sl]).astype(np.float32)
            m[f"b_k{st}"] = np.ascontiguousarray(bk[st][sl]).astype(np.float32)
            m[f"b_v{st}"] = np.ascontiguousarray(bv[st][sl]).astype(np.float32)
        m["w_ov"] = np.ascontiguousarray(wo["v"][sl, :]).astype(bf)
        m["w_oi"] = np.ascontiguousarray(wo["i"][sl, :]).astype(bf)
        in_maps.append(m)

    res = run_bass_kernel_spmd(nc, in_maps, list(range(N_CORES))).results

    ov = np.zeros((B, D, S), np.float32)
    oi = np.zeros((B, D, S), np.float32)
    for c in range(N_CORES):
        ov += res[c]["o_vis"].astype(np.float32)
        oi += res[c]["o_inf"].astype(np.float32)
    out_vis = ov.transpose(0, 2, 1) + np.asarray(b_o_vis)[None, None, :]
    out_inf = oi.transpose(0, 2, 1) + np.asarray(b_o_inf)[None, None, :]
    return (out_vis.astype(np.float32), out_inf.astype(np.float32))
